# revision 6
# baseline (speedup 1.0000x reference)
"""Trainium2 Bass kernel for nn_Attention_83004537963197.

LayerNorm -> QKV projection -> 8-head attention (head_dim=16) -> output
projection, x[16, 1024, 1024] f32.  Data-parallel over batch: 2 batches
per NeuronCore across 8 cores, no collectives.

v2 changes over the original baseline (282us):
  - softmax exp split across ScalarE (exact ACT exp) and VectorE
    (Schraudolph bit-trick exp: one tensor_scalar f32->int16 whose bit
    pattern is read back as bf16; ~3% per-element noise that cancels
    after softmax normalization).
  - LN rsqrt via int-shift + Newton on VectorE (no Sqrt on ScalarE ->
    single ACT table load for the whole kernel, no table thrashing).
  - LN center/scale on GpSimd (frees VectorE for exp).
  - softmax normalization: evac oT, DMA-gather the 4 rowsum rows,
    reciprocal_approx_fast, gpsimd partition_broadcast + gpsimd mul
    (no DRAM round-trip).
  - PSUM evacuations (xT / qk / v / proj) distributed across ScalarE
    and VectorE to balance engine load.
"""

from contextlib import ExitStack

import numpy as np
import ml_dtypes

import concourse.bass as bass
import concourse.tile as tile
from concourse import bacc, mybir
from concourse.bass_utils import run_bass_kernel_spmd

F32 = mybir.dt.float32
BF16 = mybir.dt.bfloat16
I16 = mybir.dt.int16
I32 = mybir.dt.int32

B, N, EMB = 16, 1024, 1024
HEADS, INNER = 8, 128
HD = INNER // HEADS            # 16
SCALE = INNER ** -0.5
EPS = 1e-5
NCORES = 8
NB = B // NCORES               # batches per core
P = 128
NT = EMB // P                  # 8 tiles along emb / n

Sub = mybir.AluOpType.subtract
Mult = mybir.AluOpType.mult
Add = mybir.AluOpType.add
Bypass = mybir.AluOpType.bypass
Shr = mybir.AluOpType.logical_shift_right
AF = mybir.ActivationFunctionType

# Schraudolph exp via bf16 bit pattern: exp(s) ~= bf16_bits(round(s*A + B))
A_EXP = 184.6649652337  # 128 * log2(e)
B_EXP = 16248.67        # 127*128 - 7.33 (centers the log-ratio error)
RSQRT_MAGIC = 0x5f3759df

_CACHE = {}


def _build():
    nc = bacc.Bacc(None, target_bir_lowering=False)

    xs_h = nc.declare_dram_parameter("xs", [NB, N, EMB], F32, isOutput=False)
    wqk_h = nc.declare_dram_parameter("wqk", [P, NT, 2, P], BF16, isOutput=False)
    bqk_h = nc.declare_dram_parameter("bqk", [P, 2], F32, isOutput=False)
    wv_h = nc.declare_dram_parameter("wv", [P, NT, P], BF16, isOutput=False)
    bv_h = nc.declare_dram_parameter("bv", [1, P], BF16, isOutput=False)
    wpj_h = nc.declare_dram_parameter("wproj", [P, 2, EMB], BF16, isOutput=False)
    id_h = nc.declare_dram_parameter("ident", [P, P], BF16, isOutput=False)
    out_h = nc.declare_dram_parameter("out", [NB, N, EMB], F32, isOutput=True)

    with tile.TileContext(nc) as tc, ExitStack() as ctx:
        ent = ctx.enter_context
        const = ent(tc.tile_pool(name="const", bufs=1))
        xpool = ent(tc.tile_pool(name="xpool", bufs=3))
        xt_pool = ent(tc.tile_pool(name="xtp", bufs=10))
        stat = ent(tc.tile_pool(name="stat", bufs=2))
        xT_pool = ent(tc.tile_pool(name="xT", bufs=2))
        qk_pool = ent(tc.tile_pool(name="qk", bufs=2))
        v_pool = ent(tc.tile_pool(name="vp", bufs=2))
        e_pool = ent(tc.tile_pool(name="ep", bufs=4))
        o_pool = ent(tc.tile_pool(name="op", bufs=4))
        nrm_pool = ent(tc.tile_pool(name="nrm", bufs=3))
        fin_pool = ent(tc.tile_pool(name="fin", bufs=4))
        ps_small = ent(tc.tile_pool(name="pss", bufs=2, space="PSUM"))
        ps_scores = ent(tc.tile_pool(name="psc", bufs=2, space="PSUM"))
        ps_out = ent(tc.tile_pool(name="pso", bufs=2, space="PSUM"))

        from concourse import library_config
        nc.gpsimd.load_library(library_config.proxy)

        # ---- constants ----
        wqk_sb = const.tile([P, NT, 2, P], BF16)
        nc.sync.dma_start(out=wqk_sb, in_=wqk_h[:])
        bqk_sb = const.tile([P, 2], F32)
        nc.sync.dma_start(out=bqk_sb, in_=bqk_h[:])
        wv_sb = const.tile([P, NT, P], BF16)
        nc.sync.dma_start(out=wv_sb, in_=wv_h[:])
        bv_sb = const.tile([1, P], BF16)
        nc.sync.dma_start(out=bv_sb, in_=bv_h[:])
        wpj_sb = const.tile([P, 2, EMB], BF16)
        nc.sync.dma_start(out=wpj_sb, in_=wpj_h[:])
        id_sb = const.tile([P, P], BF16)
        nc.sync.dma_start(out=id_sb, in_=id_h[:])
        eps_sb = const.tile([P, 1], F32)
        nc.vector.memset(eps_sb, EPS)
        ones1_sb = const.tile([1, P], BF16)
        nc.vector.memset(ones1_sb, 1.0)
        magic_sb = const.tile([P, NT], I32)
        nc.vector.memset(magic_sb, RSQRT_MAGIC)

        st8 = {}   # per-batch live tiles

        def emit_x_load(b, it):
            s = st8[b]
            if s.get("xt") is None:
                s["xt"] = [None] * NT
            xt = xt_pool.tile([P, EMB], F32, tag="xt")
            nc.sync.dma_start(out=xt, in_=xs_h[b, it * P:(it + 1) * P, :])
            s["xt"][it] = xt

        def emit_ln_stats(b, it):
            s = st8[b]
            if s.get("mv") is None:
                s["mv"] = stat.tile([P, NT, 2], F32, tag="mv", name=f"mv{b}")
            xt = s["xt"][it]
            st = stat.tile([P, 2, 6], F32, tag="st")
            nc.vector.bn_stats(out=st[:, 0, :], in_=xt[:, 0:512])
            nc.vector.bn_stats(out=st[:, 1, :], in_=xt[:, 512:1024])
            nc.vector.bn_aggr(out=s["mv"][:, it, :], in_=st)

        def emit_rsqrt(b):
            # rs8 = 1/sqrt(var + eps) for all 8 tiles, entirely on VectorE:
            # Quake seed (int shift) + 2 Newton iterations, FD=8.
            s = st8[b]
            mv = s["mv"]
            ve = stat.tile([P, NT], F32, tag="ve", name=f"ve{b}")
            nc.vector.tensor_scalar(
                out=ve, in0=mv[:, :, 1], scalar1=EPS, scalar2=None, op0=Add)
            ti = stat.tile([P, NT], I32, tag="ti")
            nc.vector.tensor_scalar(
                out=ti, in0=ve[:].bitcast(I32), scalar1=1, scalar2=None,
                op0=Shr)
            y0 = stat.tile([P, NT], F32, tag="y0")
            nc.vector.scalar_tensor_tensor(
                out=y0[:].bitcast(I32), in0=magic_sb, scalar=0, in1=ti,
                op0=Bypass, op1=Sub)
            y = y0
            for k in range(2):
                a = stat.tile([P, NT], F32, tag=f"nra{k}")
                nc.vector.scalar_tensor_tensor(
                    out=a, in0=y, scalar=1.0, in1=y, op0=Bypass, op1=Mult)
                bt = stat.tile([P, NT], F32, tag=f"nrb{k}")
                nc.vector.scalar_tensor_tensor(
                    out=bt, in0=a, scalar=-0.5, in1=ve, op0=Mult, op1=Mult)
                y2 = stat.tile([P, NT], F32, tag=f"nry{k}")
                nc.vector.scalar_tensor_tensor(
                    out=y2, in0=bt, scalar=1.5, in1=y, op0=Add, op1=Mult)
                y = y2
            s["rs"] = y

        def emit_center(b, it):
            # xn = (x - mu) * rs  -> bf16, on GpSimd
            s = st8[b]
            if s.get("xn") is None:
                s["xn"] = [None] * NT
            xn = xpool.tile([P, EMB], BF16, tag="xn")
            nc.gpsimd.tensor_scalar(
                out=xn, in0=s["xt"][it], scalar1=s["mv"][:, it, 0:1],
                scalar2=s["rs"][:, it:it + 1], op0=Sub, op1=Mult)
            s["xn"][it] = xn
            s["xt"][it] = None

        def emit_ln_tp(b, it):
            s = st8[b]
            if s.get("xT") is None:
                s["xT"] = xT_pool.tile([P, NT, N], BF16, tag="xTt", name="xTt")
            xT = s["xT"]
            xn = s["xn"][it]
            for eg in range(2):
                tp = ps_small.tile([P, 4, P], F32, tag="smallps")
                for kk in range(4):
                    et = 4 * eg + kk
                    nc.tensor.matmul(
                        tp[:, kk, :], xn[:, et * P:(et + 1) * P], id_sb,
                        start=True, stop=True)
                dst = xT[:, 4 * eg:4 * eg + 4, it * P:(it + 1) * P]
                if eg == 0:
                    nc.scalar.copy(out=dst, in_=tp)
                else:
                    nc.vector.tensor_copy(out=dst, in_=tp)

        def emit_qk_chunk(b, t, nt):
            # compact q^T/k^T halves; on the last nt of each t, relocate
            # head rows into the 32-aligned region layout.
            s = st8[b]
            if s.get("qkc") is None:
                s["qkc"] = qk_pool.tile([P, 2, N], BF16, tag="qkc", name="qkc")
                s["qT"] = qk_pool.tile([P, 2, N], BF16, tag="qT", name="qT")
                s["kT"] = qk_pool.tile([P, 2, N], BF16, tag="kT", name="kT")
            xT = s["xT"]
            ps = ps_small.tile([P, 512], F32, tag="smallps")
            for et in range(NT):
                nc.tensor.matmul(
                    ps, wqk_sb[:, et, t, :],
                    xT[:, et, nt * 512:(nt + 1) * 512],
                    start=(et == 0), stop=(et == NT - 1))
            nc.vector.tensor_scalar(
                out=s["qkc"][:, t, nt * 512:(nt + 1) * 512], in0=ps,
                scalar1=bqk_sb[:, t:t + 1], scalar2=None, op0=Add)
            if nt == 1:
                dst = s["qT"] if t == 0 else s["kT"]
                for h in range(HEADS):
                    r, c = h // 4, h % 4
                    nc.gpsimd.dma_start(
                        out=dst[32 * c:32 * c + HD, r, :],
                        in_=s["qkc"][HD * h:HD * (h + 1), t, :])

        def emit_v_chunk(b, jt):
            s = st8[b]
            if s.get("v") is None:
                s["v"] = v_pool.tile([P, NT, HEADS, 32], BF16, tag="vt", name="vt")
                nc.gpsimd.memset(s["v"], 0.0)
                nc.gpsimd.memset(s["v"][:, :, :, 0:1], 1.0)
            xT = s["xT"]
            ps = ps_small.tile([P, P], F32, tag="smallps")
            for et in range(NT):
                nc.tensor.matmul(
                    ps, xT[:, et, jt * P:(jt + 1) * P], wv_sb[:, et, :],
                    start=(et == 0), stop=False)
            nc.tensor.matmul(ps, ones1_sb, bv_sb, start=False, stop=True)
            nc.scalar.copy(
                out=s["v"][:, jt, :, 1:17],
                in_=ps[:].rearrange("p (h d) -> p h d", d=16))

        def emit_proj_chunk(b, it, nt):
            s = st8[b]
            ps = ps_small.tile([P, 512], F32, tag="smallps")
            for r in range(2):
                nc.tensor.matmul(
                    ps, s["o"][r][:, it * P:(it + 1) * P],
                    wpj_sb[:, r, nt * 512:(nt + 1) * 512],
                    start=(r == 0), stop=(r == 1))
            fin = fin_pool.tile([P, 512], F32, tag="fin")
            if (it + nt) % 2 == 0:
                nc.scalar.copy(out=fin, in_=ps)
            else:
                nc.vector.tensor_copy(out=fin, in_=ps)
            nc.sync.dma_start(
                out=out_h[b, it * P:(it + 1) * P, nt * 512:(nt + 1) * 512],
                in_=fin)

        def emit_proj1(b, it, nt):
            # region-0 half of the projection, stashed in SBUF bf16
            s = st8[b]
            if s.get("fin1") is None:
                s["fin1"] = fin_pool.tile([P, NT, 2, 512], BF16,
                                          tag="fin1", name="fin1", bufs=1)
            ps = ps_small.tile([P, 512], F32, tag="smallps")
            nc.tensor.matmul(
                ps, s["o"][0][:, it * P:(it + 1) * P],
                wpj_sb[:, 0, nt * 512:(nt + 1) * 512],
                start=True, stop=True)
            if (it + nt) % 2 == 0:
                nc.scalar.copy(out=s["fin1"][:, it, nt, :], in_=ps)
            else:
                nc.vector.tensor_copy(out=s["fin1"][:, it, nt, :], in_=ps)

        def emit_proj2(b, it, nt):
            s = st8[b]
            ps = ps_small.tile([P, 512], F32, tag="smallps")
            nc.tensor.matmul(
                ps, s["o"][1][:, it * P:(it + 1) * P],
                wpj_sb[:, 1, nt * 512:(nt + 1) * 512],
                start=True, stop=True)
            fin = fin_pool.tile([P, 512], F32, tag="fin")
            nc.vector.tensor_add(fin, s["fin1"][:, it, nt, :], ps)
            nc.sync.dma_start(
                out=out_h[b, it * P:(it + 1) * P, nt * 512:(nt + 1) * 512],
                in_=fin)

        def emit_normalize(b, r, ih, oT_ps):
            # divide each head-strip of oT by its softmax row-sum (rows 32c)
            s = st8[b]
            if s["o"][r] is None:
                s["o"][r] = o_pool.tile([P, N], BF16, tag="oT", name="oT")
            i0 = ih * 512
            of = nrm_pool.tile([P, 512], F32, tag="of")
            nc.vector.tensor_copy(out=of, in_=oT_ps)
            rep = nrm_pool.tile([P, 512], F32, tag="rep")
            nc.vector.stream_shuffle(out=rep, in_=of, mask=[0] * 32)
            rrep = nrm_pool.tile([P, 512], F32, tag="rrep")
            nc.vector.reciprocal_approx_fast(out=rrep, in_=rep)
            nc.gpsimd.tensor_tensor(
                out=s["o"][r][:, i0:i0 + 512], in0=of, in1=rrep, op=Mult)

        def emit_attention(b, fillers, rate=2):
            s = st8[b]
            s["o"] = [None, None]
            slot = [0]
            eslot = [0]

            def maybe_fill():
                slot[0] += 1
                if fillers and (rate > 0 and slot[0] % rate == 0 or
                                (rate == 0 and (slot[0] % 2 == 0 or
                                                slot[0] > 32))):
                    f = fillers.pop(0)
                    if f is not None:
                        f()

            for r in range(2):
                for ih in range(2):
                    oT_ps = ps_out.tile([P, 512], F32, tag="oTps")
                    i0 = ih * 512
                    for cp in range(2):
                        c0 = 2 * cp
                        for jt in range(NT):
                            E = e_pool.tile([P, 2, 512], BF16, tag="E")
                            sc = ps_scores.tile([P, 2, 512], F32, tag="sc")
                            for ci in range(2):
                                c = c0 + ci
                                nc.tensor.matmul(
                                    sc[:, ci, :],
                                    s["kT"][32 * c:32 * c + 16, r,
                                            jt * P:(jt + 1) * P],
                                    s["qT"][32 * c:32 * c + 16, r,
                                            i0:i0 + 512],
                                    start=True, stop=True,
                                    tile_position=(32 * c, 0))
                            eslot[0] += 1
                            if eslot[0] % 3 == 2:
                                nc.vector.tensor_scalar(
                                    out=E[:].bitcast(I16), in0=sc,
                                    scalar1=A_EXP, scalar2=B_EXP,
                                    op0=Mult, op1=Add)
                            else:
                                nc.scalar.activation(out=E, in_=sc, func=AF.Exp)
                            for ci in range(2):
                                c = c0 + ci
                                h = 4 * r + c
                                nc.tensor.matmul(
                                    oT_ps[32 * c:32 * c + 32, :],
                                    s["v"][:, jt, h, :], E[:, ci, :],
                                    start=(jt == 0), stop=(jt == NT - 1),
                                    tile_position=(0, 32 * c))
                            maybe_fill()
                    emit_normalize(b, r, ih, oT_ps)

        # ---------- schedule ----------
        st8[0] = {}
        st8[1] = {}
        # preload the exp table while the ramp runs
        dummy = stat.tile([P, 1], F32, tag="dummy")
        nc.scalar.activation(out=dummy, in_=eps_sb, func=AF.Exp)

        def ab_order(b):
            out = []
            for it in range(NT):
                out.append(lambda it=it: emit_x_load(b, it))
                out.append(lambda it=it: emit_ln_stats(b, it))
            out.append(lambda: emit_rsqrt(b))
            for it in range(4):
                out.append(lambda it=it: emit_center(b, it))
                out.append(lambda it=it: emit_ln_tp(b, it))
                out.append(lambda it=it: emit_v_chunk(b, it))
            out.append(lambda: emit_qk_chunk(b, 0, 0))
            out.append(lambda: emit_qk_chunk(b, 1, 0))
            for it in range(4, NT):
                out.append(lambda it=it: emit_center(b, it))
                out.append(lambda it=it: emit_ln_tp(b, it))
                out.append(lambda it=it: emit_v_chunk(b, it))
            out.append(lambda: emit_qk_chunk(b, 0, 1))
            out.append(lambda: emit_qk_chunk(b, 1, 1))
            return out

        for f in ab_order(0):
            f()

        fill_b1 = ab_order(1)
        emit_attention(0, fill_b1)
        for f in fill_b1:
            f()

        fill_a1 = (
            [lambda it=it, nt=nt: emit_proj_chunk(0, it, nt)
             for it in range(NT) for nt in range(2)]
            + [lambda it=it, nt=nt: emit_proj1(1, it, nt)
               for it in range(4) for nt in range(2)]
            + [lambda it=it, nt=nt: emit_proj1(1, it, nt)
               for it in range(4, NT) for nt in range(2)]
            + [lambda it=it, nt=nt: emit_proj2(1, it, nt)
               for it in range(4) for nt in range(2)]
        )
        emit_attention(1, fill_a1, rate=0)
        for f in fill_a1:
            if f is not None:
                f()
        for it in range(4, NT):
            for nt in range(2):
                emit_proj2(1, it, nt)

    nc.finalize()
    return nc


def _prep_weights(gamma, beta, w_qkv, w_proj, b_proj):
    gamma = gamma.astype(np.float64)
    beta = beta.astype(np.float64)
    w_qkv = w_qkv.astype(np.float64)
    w_proj = w_proj.astype(np.float64)
    b_proj = b_proj.astype(np.float64)

    wg = w_qkv * gamma[:, None]
    bias = beta @ w_qkv                   # [384]

    # compact q/k: tile t=0 -> q (SCALE folded), t=1 -> k
    wqk = np.zeros((EMB, 2, P), dtype=np.float64)
    wqk[:, 0, :] = wg[:, :INNER] * SCALE
    wqk[:, 1, :] = wg[:, INNER:2 * INNER]
    bqk = np.zeros((P, 2), dtype=np.float64)
    bqk[:, 0] = bias[:INNER] * SCALE
    bqk[:, 1] = bias[INNER:2 * INNER]
    wqk_t = wqk.reshape(NT, P, 2, P).transpose(1, 0, 2, 3)  # [P, NT, 2, P]

    wv = wg[:, 2 * INNER:3 * INNER].reshape(NT, P, P).transpose(1, 0, 2)
    bv = bias[2 * INNER:3 * INNER].reshape(1, P)

    # o^T row mapping: 32c = ones/rowsum row, 32c+1+d = head (4r+c) dim d
    wpj = np.zeros((P, 2, EMB), dtype=np.float64)
    for r in range(2):
        for c in range(4):
            h = 4 * r + c
            wpj[32 * c + 1:32 * c + 1 + HD, r, :] = \
                w_proj[h * HD:(h + 1) * HD, :]
    wpj[0, 0, :] = b_proj

    bf = ml_dtypes.bfloat16
    return {
        "wqk": np.ascontiguousarray(wqk_t).astype(bf),
        "bqk": np.ascontiguousarray(bqk).astype(np.float32),
        "wv": np.ascontiguousarray(wv).astype(bf),
        "bv": np.ascontiguousarray(bv).astype(bf),
        "wproj": np.ascontiguousarray(wpj).astype(bf),
        "ident": np.eye(P, dtype=np.float32).astype(bf),
    }


def kernel(x, gamma, beta, w_qkv, w_proj, b_proj):
    if "nc" not in _CACHE:
        _CACHE["nc"] = _build()
    nc = _CACHE["nc"]

    w = _prep_weights(gamma, beta, w_qkv, w_proj, b_proj)
    x = np.asarray(x, dtype=np.float32)
    in_maps = []
    for i in range(NCORES):
        m = {"xs": np.ascontiguousarray(x[i * NB:(i + 1) * NB])}
        m.update(w)
        in_maps.append(m)

    res = run_bass_kernel_spmd(nc, in_maps, core_ids=list(range(NCORES)))
    out = np.concatenate([res.results[i]["out"] for i in range(NCORES)], axis=0)
    return out.astype(np.float32)


# revision 7
# speedup vs baseline: 1.3113x; 1.3113x over previous
"""Trainium2 Bass kernel for nn_Attention_83004537963197.

LayerNorm -> QKV projection -> 8-head attention (head_dim=16) -> output
projection, x[16, 1024, 1024] f32.  Data-parallel over batch: 2 batches
per NeuronCore across 8 cores, no collectives.

v3 design:
  - softmax exp split across ScalarE (exact ACT exp, ~2/3) and VectorE
    (Schraudolph bit-trick exp: one tensor_scalar f32->int16 whose bit
    pattern is read back as bf16; ~3% per-element noise that cancels
    after softmax normalization).
  - LN rsqrt via int-shift + Newton on VectorE (no Sqrt on ScalarE ->
    single ACT table load for the whole kernel, no table thrashing).
  - softmax normalization: evac oT (ScalarE), stream_shuffle broadcast
    of the rowsum rows (VectorE), reciprocal_approx_fast (VectorE),
    normalize multiply on GpSimd.
  - attention groups ordered ih-outer so both head-region halves of
    each query range finish mid-attention; the output projection runs
    as plain 2-matmul PSUM-accumulated chunks inside the attention of
    the other batch (no stashed partial projections).
  - PSUM evacuations (xT / qk / v / proj) distributed across ScalarE
    and VectorE to balance engine load; qk bias added via a K=1 matmul.
"""

from contextlib import ExitStack

import numpy as np
import ml_dtypes

import concourse.bass as bass
import concourse.tile as tile
from concourse import bacc, mybir
from concourse.bass_utils import run_bass_kernel_spmd

F32 = mybir.dt.float32
BF16 = mybir.dt.bfloat16
I16 = mybir.dt.int16
I32 = mybir.dt.int32

B, N, EMB = 16, 1024, 1024
HEADS, INNER = 8, 128
HD = INNER // HEADS            # 16
SCALE = INNER ** -0.5
EPS = 1e-5
NCORES = 8
NB = B // NCORES               # batches per core
P = 128
NT = EMB // P                  # 8 tiles along emb / n

Sub = mybir.AluOpType.subtract
Mult = mybir.AluOpType.mult
Add = mybir.AluOpType.add
Bypass = mybir.AluOpType.bypass
Shr = mybir.AluOpType.logical_shift_right
AF = mybir.ActivationFunctionType

# Schraudolph exp via bf16 bit pattern: exp(s) ~= bf16_bits(round(s*A + B))
A_EXP = 184.6649652337  # 128 * log2(e)
B_EXP = 16248.67        # 127*128 - 7.33 (centers the log-ratio error)
RSQRT_MAGIC = 0x5f3759df

_CACHE = {}


def _build():
    nc = bacc.Bacc(None, target_bir_lowering=False)

    xs_h = nc.declare_dram_parameter("xs", [NB, N, EMB], F32, isOutput=False)
    wqk_h = nc.declare_dram_parameter("wqk", [P, NT, 2, P], BF16, isOutput=False)
    bqk_h = nc.declare_dram_parameter("bqk", [1, 2, P], BF16, isOutput=False)
    wv_h = nc.declare_dram_parameter("wv", [P, NT, P], BF16, isOutput=False)
    bv_h = nc.declare_dram_parameter("bv", [1, P], BF16, isOutput=False)
    wpj_h = nc.declare_dram_parameter("wproj", [P, 2, EMB], BF16, isOutput=False)
    id_h = nc.declare_dram_parameter("ident", [P, P], BF16, isOutput=False)
    out_h = nc.declare_dram_parameter("out", [NB, N, EMB], F32, isOutput=True)

    with tile.TileContext(nc) as tc, ExitStack() as ctx:
        ent = ctx.enter_context
        const = ent(tc.tile_pool(name="const", bufs=1))
        xpool = ent(tc.tile_pool(name="xpool", bufs=3))
        xt_pool = ent(tc.tile_pool(name="xtp", bufs=10))
        stat = ent(tc.tile_pool(name="stat", bufs=2))
        xT_pool = ent(tc.tile_pool(name="xT", bufs=2))
        qk_pool = ent(tc.tile_pool(name="qk", bufs=2))
        v_pool = ent(tc.tile_pool(name="vp", bufs=2))
        e_pool = ent(tc.tile_pool(name="ep", bufs=4))
        o_pool = ent(tc.tile_pool(name="op", bufs=4))
        nrm_pool = ent(tc.tile_pool(name="nrm", bufs=3))
        fin_pool = ent(tc.tile_pool(name="fin", bufs=4))
        ps_small = ent(tc.tile_pool(name="pss", bufs=2, space="PSUM"))
        ps_scores = ent(tc.tile_pool(name="psc", bufs=2, space="PSUM"))
        ps_out = ent(tc.tile_pool(name="pso", bufs=2, space="PSUM"))

        from concourse import library_config
        nc.gpsimd.load_library(library_config.proxy)

        # ---- constants ----
        wqk_sb = const.tile([P, NT, 2, P], BF16)
        nc.sync.dma_start(out=wqk_sb, in_=wqk_h[:])
        bqk_sb = const.tile([1, 2, P], BF16)
        nc.sync.dma_start(out=bqk_sb, in_=bqk_h[:])
        wv_sb = const.tile([P, NT, P], BF16)
        nc.sync.dma_start(out=wv_sb, in_=wv_h[:])
        bv_sb = const.tile([1, P], BF16)
        nc.sync.dma_start(out=bv_sb, in_=bv_h[:])
        wpj_sb = const.tile([P, 2, EMB], BF16)
        nc.sync.dma_start(out=wpj_sb, in_=wpj_h[:])
        id_sb = const.tile([P, P], BF16)
        nc.sync.dma_start(out=id_sb, in_=id_h[:])
        ones512_sb = const.tile([1, 512], BF16)
        nc.vector.memset(ones512_sb, 1.0)
        ones1_sb = const.tile([1, P], BF16)
        nc.vector.memset(ones1_sb, 1.0)
        magic_sb = const.tile([P, NT], I32)
        nc.vector.memset(magic_sb, RSQRT_MAGIC)
        eps_sb = const.tile([P, 1], F32)
        nc.vector.memset(eps_sb, EPS)

        st8 = {}   # per-batch live tiles

        def emit_x_load(b, it):
            s = st8[b]
            if s.get("xt") is None:
                s["xt"] = [None] * NT
            xt = xt_pool.tile([P, EMB], F32, tag="xt")
            nc.sync.dma_start(out=xt, in_=xs_h[b, it * P:(it + 1) * P, :])
            s["xt"][it] = xt

        def emit_ln_stats(b, it):
            s = st8[b]
            if s.get("mv") is None:
                s["mv"] = stat.tile([P, NT, 2], F32, tag="mv", name=f"mv{b}")
            xt = s["xt"][it]
            st = stat.tile([P, 2, 6], F32, tag="st")
            nc.vector.bn_stats(out=st[:, 0, :], in_=xt[:, 0:512])
            nc.vector.bn_stats(out=st[:, 1, :], in_=xt[:, 512:1024])
            nc.vector.bn_aggr(out=s["mv"][:, it, :], in_=st)

        def emit_rsqrt(b):
            # rs8 = 1/sqrt(var + eps) for all 8 tiles, entirely on VectorE:
            # Quake seed (int shift) + 2 Newton iterations, FD=8.
            s = st8[b]
            mv = s["mv"]
            ve = stat.tile([P, NT], F32, tag="ve", name=f"ve{b}")
            nc.vector.tensor_scalar(
                out=ve, in0=mv[:, :, 1], scalar1=EPS, scalar2=None, op0=Add)
            ti = stat.tile([P, NT], I32, tag="ti")
            nc.vector.tensor_scalar(
                out=ti, in0=ve[:].bitcast(I32), scalar1=1, scalar2=None,
                op0=Shr)
            y0 = stat.tile([P, NT], F32, tag="y0")
            nc.vector.scalar_tensor_tensor(
                out=y0[:].bitcast(I32), in0=magic_sb, scalar=0, in1=ti,
                op0=Bypass, op1=Sub)
            y = y0
            for k in range(2):
                a = stat.tile([P, NT], F32, tag=f"nra{k}")
                nc.vector.scalar_tensor_tensor(
                    out=a, in0=y, scalar=1.0, in1=y, op0=Bypass, op1=Mult)
                bt = stat.tile([P, NT], F32, tag=f"nrb{k}")
                nc.vector.scalar_tensor_tensor(
                    out=bt, in0=a, scalar=-0.5, in1=ve, op0=Mult, op1=Mult)
                y2 = stat.tile([P, NT], F32, tag=f"nry{k}")
                nc.vector.scalar_tensor_tensor(
                    out=y2, in0=bt, scalar=1.5, in1=y, op0=Add, op1=Mult)
                y = y2
            s["rs"] = y

        def emit_center(b, it):
            # xn = (x - mu) * rs  -> bf16 (VectorE 2x mode, f32 single-src)
            s = st8[b]
            if s.get("xn") is None:
                s["xn"] = [None] * NT
            xn = xpool.tile([P, EMB], BF16, tag="xn")
            nc.vector.tensor_scalar(
                out=xn, in0=s["xt"][it], scalar1=s["mv"][:, it, 0:1],
                scalar2=s["rs"][:, it:it + 1], op0=Sub, op1=Mult)
            s["xn"][it] = xn
            s["xt"][it] = None

        def emit_ln_tp(b, it):
            s = st8[b]
            if s.get("xT") is None:
                s["xT"] = xT_pool.tile([P, NT, N], BF16, tag="xTt", name="xTt")
            xT = s["xT"]
            xn = s["xn"][it]
            for eg in range(2):
                tp = ps_small.tile([P, 4, P], F32, tag="smallps")
                for kk in range(4):
                    et = 4 * eg + kk
                    nc.tensor.matmul(
                        tp[:, kk, :], xn[:, et * P:(et + 1) * P], id_sb,
                        start=True, stop=True)
                dst = xT[:, 4 * eg:4 * eg + 4, it * P:(it + 1) * P]
                if eg == 0:
                    nc.scalar.copy(out=dst, in_=tp)
                else:
                    nc.vector.tensor_copy(out=dst, in_=tp)

        def emit_qk_chunk(b, t, nt):
            # compact q^T/k^T halves; on the last nt of each t, relocate
            # head rows into the 32-aligned region layout.
            s = st8[b]
            if s.get("qkc") is None:
                s["qkc"] = qk_pool.tile([P, 2, N], BF16, tag="qkc", name="qkc")
                s["qT"] = qk_pool.tile([P, 2, N], BF16, tag="qT", name="qT")
                s["kT"] = qk_pool.tile([P, 2, N], BF16, tag="kT", name="kT")
            xT = s["xT"]
            ps = ps_small.tile([P, 512], F32, tag="smallps")
            for et in range(NT):
                nc.tensor.matmul(
                    ps, wqk_sb[:, et, t, :],
                    xT[:, et, nt * 512:(nt + 1) * 512],
                    start=(et == 0), stop=False)
            nc.tensor.matmul(
                ps, bqk_sb[:, t, :], ones512_sb, start=False, stop=True)
            nc.scalar.copy(
                out=s["qkc"][:, t, nt * 512:(nt + 1) * 512], in_=ps)
            if nt == 1:
                dst = s["qT"] if t == 0 else s["kT"]
                for h in range(HEADS):
                    r, c = h // 4, h % 4
                    nc.gpsimd.dma_start(
                        out=dst[32 * c:32 * c + HD, r, :],
                        in_=s["qkc"][HD * h:HD * (h + 1), t, :])

        def emit_v_chunk(b, jt):
            s = st8[b]
            if s.get("v") is None:
                s["v"] = v_pool.tile([P, NT, HEADS, 32], BF16, tag="vt", name="vt")
                nc.gpsimd.memset(s["v"], 0.0)
                nc.gpsimd.memset(s["v"][:, :, :, 0:1], 1.0)
            xT = s["xT"]
            ps = ps_small.tile([P, P], F32, tag="smallps")
            for et in range(NT):
                nc.tensor.matmul(
                    ps, xT[:, et, jt * P:(jt + 1) * P], wv_sb[:, et, :],
                    start=(et == 0), stop=False)
            nc.tensor.matmul(ps, ones1_sb, bv_sb, start=False, stop=True)
            nc.scalar.copy(
                out=s["v"][:, jt, :, 1:17],
                in_=ps[:].rearrange("p (h d) -> p h d", d=16))

        def emit_proj_chunk(b, it, nt):
            s = st8[b]
            ps = ps_small.tile([P, 512], F32, tag="smallps")
            for r in range(2):
                nc.tensor.matmul(
                    ps, s["o"][r][:, it * P:(it + 1) * P],
                    wpj_sb[:, r, nt * 512:(nt + 1) * 512],
                    start=(r == 0), stop=(r == 1))
            fin = fin_pool.tile([P, 512], F32, tag="fin")
            if (it + nt) % 2 == 0:
                nc.scalar.copy(out=fin, in_=ps)
            else:
                nc.vector.tensor_copy(out=fin, in_=ps)
            nc.sync.dma_start(
                out=out_h[b, it * P:(it + 1) * P, nt * 512:(nt + 1) * 512],
                in_=fin)

        def emit_normalize(b, r, ih, oT_ps):
            # divide each head-strip of oT by its softmax row-sum (rows 32c)
            s = st8[b]
            if s["o"][r] is None:
                s["o"][r] = o_pool.tile([P, N], BF16, tag="oT", name="oT")
            i0 = ih * 512
            of = nrm_pool.tile([P, 512], F32, tag="of")
            nc.scalar.copy(out=of, in_=oT_ps)
            rep = nrm_pool.tile([P, 512], F32, tag="rep")
            nc.vector.stream_shuffle(out=rep, in_=of, mask=[0] * 32)
            rrep = nrm_pool.tile([P, 512], F32, tag="rrep")
            nc.vector.reciprocal_approx_fast(out=rrep, in_=rep)
            nc.gpsimd.tensor_tensor(
                out=s["o"][r][:, i0:i0 + 512], in0=of, in1=rrep, op=Mult)

        def emit_attention(b, fillers, rate=2):
            s = st8[b]
            s["o"] = [None, None]
            slot = [0]
            eslot = [0]

            def maybe_fill():
                slot[0] += 1
                if fillers and (rate == 1 or
                                (rate > 1 and slot[0] % rate == 0)):
                    f = fillers.pop(0)
                    if f is not None:
                        f()

            for ih in range(2):
                for r in range(2):
                    oT_ps = ps_out.tile([P, 512], F32, tag="oTps")
                    i0 = ih * 512
                    for cp in range(2):
                        c0 = 2 * cp
                        for jt in range(NT):
                            E = e_pool.tile([P, 2, 512], BF16, tag="E")
                            sc = ps_scores.tile([P, 2, 512], F32, tag="sc")
                            for ci in range(2):
                                c = c0 + ci
                                nc.tensor.matmul(
                                    sc[:, ci, :],
                                    s["kT"][32 * c:32 * c + 16, r,
                                            jt * P:(jt + 1) * P],
                                    s["qT"][32 * c:32 * c + 16, r,
                                            i0:i0 + 512],
                                    start=True, stop=True,
                                    tile_position=(32 * c, 0))
                            eslot[0] += 1
                            if eslot[0] % 3 == 2:
                                nc.vector.tensor_scalar(
                                    out=E[:].bitcast(I16), in0=sc,
                                    scalar1=A_EXP, scalar2=B_EXP,
                                    op0=Mult, op1=Add)
                            else:
                                nc.scalar.activation(out=E, in_=sc, func=AF.Exp)
                            for ci in range(2):
                                c = c0 + ci
                                h = 4 * r + c
                                nc.tensor.matmul(
                                    oT_ps[32 * c:32 * c + 32, :],
                                    s["v"][:, jt, h, :], E[:, ci, :],
                                    start=(jt == 0), stop=(jt == NT - 1),
                                    tile_position=(0, 32 * c))
                            maybe_fill()
                    emit_normalize(b, r, ih, oT_ps)

        # ---------- schedule ----------
        st8[0] = {}
        st8[1] = {}
        # preload the exp table while the ramp runs
        dummy = stat.tile([P, 1], F32, tag="dummy")
        nc.scalar.activation(out=dummy, in_=eps_sb, func=AF.Exp)

        def ab_order(b):
            out = []
            for it in range(NT):
                out.append(lambda it=it: emit_x_load(b, it))
                out.append(lambda it=it: emit_ln_stats(b, it))
            out.append(lambda: emit_rsqrt(b))
            for it in range(4):
                out.append(lambda it=it: emit_center(b, it))
                out.append(lambda it=it: emit_ln_tp(b, it))
                out.append(lambda it=it: emit_v_chunk(b, it))
            out.append(lambda: emit_qk_chunk(b, 0, 0))
            out.append(lambda: emit_qk_chunk(b, 1, 0))
            for it in range(4, NT):
                out.append(lambda it=it: emit_center(b, it))
                out.append(lambda it=it: emit_ln_tp(b, it))
                out.append(lambda it=it: emit_v_chunk(b, it))
            out.append(lambda: emit_qk_chunk(b, 0, 1))
            out.append(lambda: emit_qk_chunk(b, 1, 1))
            return out

        for f in ab_order(0):
            f()

        fill_b1 = ab_order(1)
        emit_attention(0, fill_b1, rate=1)
        for f in fill_b1:
            f()

        # proj(0) fills the first half of attention(1); proj(1) columns
        # 0..511 (it 0..3) become available after the ih=0 groups.
        fill_a1 = (
            [lambda it=it, nt=nt: emit_proj_chunk(0, it, nt)
             for it in range(NT) for nt in range(2)]
            + [lambda it=it, nt=nt: emit_proj_chunk(1, it, nt)
               for it in range(4) for nt in range(2)]
        )
        emit_attention(1, fill_a1, rate=2)
        for f in fill_a1:
            f()
        for it in range(4, NT):
            for nt in range(2):
                emit_proj_chunk(1, it, nt)

    nc.finalize()
    return nc


def _prep_weights(gamma, beta, w_qkv, w_proj, b_proj):
    gamma = gamma.astype(np.float64)
    beta = beta.astype(np.float64)
    w_qkv = w_qkv.astype(np.float64)
    w_proj = w_proj.astype(np.float64)
    b_proj = b_proj.astype(np.float64)

    wg = w_qkv * gamma[:, None]
    bias = beta @ w_qkv                   # [384]

    # compact q/k: tile t=0 -> q (SCALE folded), t=1 -> k
    wqk = np.zeros((EMB, 2, P), dtype=np.float64)
    wqk[:, 0, :] = wg[:, :INNER] * SCALE
    wqk[:, 1, :] = wg[:, INNER:2 * INNER]
    bqk = np.zeros((1, 2, P), dtype=np.float64)
    bqk[0, 0, :] = bias[:INNER] * SCALE
    bqk[0, 1, :] = bias[INNER:2 * INNER]
    wqk_t = wqk.reshape(NT, P, 2, P).transpose(1, 0, 2, 3)  # [P, NT, 2, P]

    wv = wg[:, 2 * INNER:3 * INNER].reshape(NT, P, P).transpose(1, 0, 2)
    bv = bias[2 * INNER:3 * INNER].reshape(1, P)

    # o^T row mapping: 32c = ones/rowsum row, 32c+1+d = head (4r+c) dim d
    wpj = np.zeros((P, 2, EMB), dtype=np.float64)
    for r in range(2):
        for c in range(4):
            h = 4 * r + c
            wpj[32 * c + 1:32 * c + 1 + HD, r, :] = \
                w_proj[h * HD:(h + 1) * HD, :]
    wpj[0, 0, :] = b_proj

    bf = ml_dtypes.bfloat16
    return {
        "wqk": np.ascontiguousarray(wqk_t).astype(bf),
        "bqk": np.ascontiguousarray(bqk).astype(bf),
        "wv": np.ascontiguousarray(wv).astype(bf),
        "bv": np.ascontiguousarray(bv).astype(bf),
        "wproj": np.ascontiguousarray(wpj).astype(bf),
        "ident": np.eye(P, dtype=np.float32).astype(bf),
    }


def kernel(x, gamma, beta, w_qkv, w_proj, b_proj):
    if "nc" not in _CACHE:
        _CACHE["nc"] = _build()
    nc = _CACHE["nc"]

    w = _prep_weights(gamma, beta, w_qkv, w_proj, b_proj)
    x = np.asarray(x, dtype=np.float32)
    in_maps = []
    for i in range(NCORES):
        m = {"xs": np.ascontiguousarray(x[i * NB:(i + 1) * NB])}
        m.update(w)
        in_maps.append(m)

    res = run_bass_kernel_spmd(nc, in_maps, core_ids=list(range(NCORES)))
    out = np.concatenate([res.results[i]["out"] for i in range(NCORES)], axis=0)
    return out.astype(np.float32)


# revision 11
# speedup vs baseline: 1.7042x; 1.2996x over previous
"""Trainium2 Bass kernel for nn_Attention_83004537963197.

LayerNorm -> QKV projection -> 8-head attention (head_dim=16) -> output
projection, x[16, 1024, 1024] f32.  Data-parallel over batch: 2 batches
per NeuronCore across 8 cores, no collectives.

v3 design:
  - softmax exp split across ScalarE (exact ACT exp, ~2/3) and VectorE
    (Schraudolph bit-trick exp: one tensor_scalar f32->int16 whose bit
    pattern is read back as bf16; ~3% per-element noise that cancels
    after softmax normalization).
  - LN rsqrt via int-shift + Newton on VectorE (no Sqrt on ScalarE ->
    single ACT table load for the whole kernel, no table thrashing).
  - softmax normalization: evac oT (ScalarE), stream_shuffle broadcast
    of the rowsum rows (VectorE), reciprocal_approx_fast (VectorE),
    normalize multiply on GpSimd.
  - attention groups ordered ih-outer so both head-region halves of
    each query range finish mid-attention; the output projection runs
    as plain 2-matmul PSUM-accumulated chunks inside the attention of
    the other batch (no stashed partial projections).
  - PSUM evacuations (xT / qk / v / proj) distributed across ScalarE
    and VectorE to balance engine load; qk bias added via a K=1 matmul.
"""

from contextlib import ExitStack

import numpy as np
import ml_dtypes

import concourse.bass as bass
import concourse.tile as tile
from concourse import bacc, mybir
from concourse.bass_utils import run_bass_kernel_spmd

F32 = mybir.dt.float32
BF16 = mybir.dt.bfloat16
I16 = mybir.dt.int16
I32 = mybir.dt.int32

B, N, EMB = 16, 1024, 1024
HEADS, INNER = 8, 128
HD = INNER // HEADS            # 16
SCALE = INNER ** -0.5
EPS = 1e-5
NCORES = 8
NB = B // NCORES               # batches per core
P = 128
NT = EMB // P                  # 8 tiles along emb / n

Sub = mybir.AluOpType.subtract
Mult = mybir.AluOpType.mult
Add = mybir.AluOpType.add
Bypass = mybir.AluOpType.bypass
Shr = mybir.AluOpType.logical_shift_right
AF = mybir.ActivationFunctionType

# Schraudolph exp via bf16 bit pattern: exp(s) ~= bf16_bits(round(s*A + B))
A_EXP = 184.6649652337  # 128 * log2(e)
B_EXP = 16248.67        # 127*128 - 7.33 (centers the log-ratio error)
RSQRT_MAGIC = 0x5f3759df

_CACHE = {}


def _build():
    nc = bacc.Bacc(None, target_bir_lowering=False)

    xs_h = nc.declare_dram_parameter("xs", [NB, N, EMB], F32, isOutput=False)
    wqk_h = nc.declare_dram_parameter("wqk", [P, NT, 2, P], BF16, isOutput=False)
    bqk_h = nc.declare_dram_parameter("bqk", [1, 2, P], BF16, isOutput=False)
    wv_h = nc.declare_dram_parameter("wv", [P, NT, P], BF16, isOutput=False)
    bv_h = nc.declare_dram_parameter("bv", [1, P], BF16, isOutput=False)
    wpj_h = nc.declare_dram_parameter("wproj", [P, 2, EMB], BF16, isOutput=False)
    id_h = nc.declare_dram_parameter("ident", [P, P], BF16, isOutput=False)
    out_h = nc.declare_dram_parameter("out", [NB, N, EMB], F32, isOutput=True)

    with tile.TileContext(nc) as tc, ExitStack() as ctx:
        ent = ctx.enter_context
        const = ent(tc.tile_pool(name="const", bufs=1))
        xpool = ent(tc.tile_pool(name="xpool", bufs=3))
        xt_pool = ent(tc.tile_pool(name="xtp", bufs=10))
        stat = ent(tc.tile_pool(name="stat", bufs=2))
        xT_pool = ent(tc.tile_pool(name="xT", bufs=2))
        qk_pool = ent(tc.tile_pool(name="qk", bufs=2))
        v_pool = ent(tc.tile_pool(name="vp", bufs=2))
        e_pool = ent(tc.tile_pool(name="ep", bufs=4))
        o_pool = ent(tc.tile_pool(name="op", bufs=4))
        nrm_pool = ent(tc.tile_pool(name="nrm", bufs=3))
        fin_pool = ent(tc.tile_pool(name="fin", bufs=4))
        ps_small = ent(tc.tile_pool(name="pss", bufs=2, space="PSUM"))
        ps_scores = ent(tc.tile_pool(name="psc", bufs=2, space="PSUM"))
        ps_out = ent(tc.tile_pool(name="pso", bufs=2, space="PSUM"))

        from concourse import library_config
        nc.gpsimd.load_library(library_config.proxy)

        # ---- constants ----
        wqk_sb = const.tile([P, NT, 2, P], BF16)
        nc.sync.dma_start(out=wqk_sb, in_=wqk_h[:])
        bqk_sb = const.tile([1, 2, P], BF16)
        nc.sync.dma_start(out=bqk_sb, in_=bqk_h[:])
        wv_sb = const.tile([P, NT, P], BF16)
        nc.sync.dma_start(out=wv_sb, in_=wv_h[:])
        bv_sb = const.tile([1, P], BF16)
        nc.sync.dma_start(out=bv_sb, in_=bv_h[:])
        wpj_sb = const.tile([P, 2, EMB], BF16)
        nc.sync.dma_start(out=wpj_sb, in_=wpj_h[:])
        id_sb = const.tile([P, P], BF16)
        nc.sync.dma_start(out=id_sb, in_=id_h[:])
        ones512_sb = const.tile([1, 512], BF16)
        nc.vector.memset(ones512_sb, 1.0)
        ones1_sb = const.tile([1, P], BF16)
        nc.vector.memset(ones1_sb, 1.0)
        magic_sb = const.tile([P, NT], I32)
        nc.vector.memset(magic_sb, RSQRT_MAGIC)
        eps_sb = const.tile([P, 1], F32)
        nc.vector.memset(eps_sb, EPS)

        st8 = {}   # per-batch live tiles

        def emit_x_load(b, it):
            s = st8[b]
            if s.get("xt") is None:
                s["xt"] = [None] * NT
            xt = xt_pool.tile([P, EMB], F32, tag="xt")
            nc.sync.dma_start(out=xt, in_=xs_h[b, it * P:(it + 1) * P, :])
            s["xt"][it] = xt

        def emit_ln_stats(b, it):
            s = st8[b]
            if s.get("mv") is None:
                s["mv"] = stat.tile([P, NT, 2], F32, tag="mv", name=f"mv{b}")
            xt = s["xt"][it]
            st = stat.tile([P, 2, 6], F32, tag="st")
            nc.vector.bn_stats(out=st[:, 0, :], in_=xt[:, 0:512])
            nc.vector.bn_stats(out=st[:, 1, :], in_=xt[:, 512:1024])
            nc.vector.bn_aggr(out=s["mv"][:, it, :], in_=st)

        def emit_rsqrt(b):
            # rs8 = 1/sqrt(var + eps) for all 8 tiles, entirely on VectorE:
            # Quake seed (int shift) + 2 Newton iterations, FD=8.
            s = st8[b]
            mv = s["mv"]
            ve = stat.tile([P, NT], F32, tag="ve", name=f"ve{b}")
            nc.vector.tensor_scalar(
                out=ve, in0=mv[:, :, 1], scalar1=EPS, scalar2=None, op0=Add)
            ti = stat.tile([P, NT], I32, tag="ti")
            nc.vector.tensor_scalar(
                out=ti, in0=ve[:].bitcast(I32), scalar1=1, scalar2=None,
                op0=Shr)
            y0 = stat.tile([P, NT], F32, tag="y0")
            nc.vector.scalar_tensor_tensor(
                out=y0[:].bitcast(I32), in0=magic_sb, scalar=0, in1=ti,
                op0=Bypass, op1=Sub)
            y = y0
            for k in range(2):
                a = stat.tile([P, NT], F32, tag=f"nra{k}")
                nc.vector.scalar_tensor_tensor(
                    out=a, in0=y, scalar=1.0, in1=y, op0=Bypass, op1=Mult)
                bt = stat.tile([P, NT], F32, tag=f"nrb{k}")
                nc.vector.scalar_tensor_tensor(
                    out=bt, in0=a, scalar=-0.5, in1=ve, op0=Mult, op1=Mult)
                y2 = stat.tile([P, NT], F32, tag=f"nry{k}")
                nc.vector.scalar_tensor_tensor(
                    out=y2, in0=bt, scalar=1.5, in1=y, op0=Add, op1=Mult)
                y = y2
            s["rs"] = y

        def emit_center(b, it):
            # xn = (x - mu) * rs  -> bf16 (VectorE 2x mode, f32 single-src)
            s = st8[b]
            if s.get("xn") is None:
                s["xn"] = [None] * NT
            xn = xpool.tile([P, EMB], BF16, tag="xn")
            nc.vector.tensor_scalar(
                out=xn, in0=s["xt"][it], scalar1=s["mv"][:, it, 0:1],
                scalar2=s["rs"][:, it:it + 1], op0=Sub, op1=Mult)
            s["xn"][it] = xn
            s["xt"][it] = None

        def emit_ln_tp(b, it):
            s = st8[b]
            if s.get("xT") is None:
                s["xT"] = xT_pool.tile([P, NT, N], BF16, tag="xTt", name="xTt")
            xT = s["xT"]
            xn = s["xn"][it]
            for eg in range(2):
                tp = ps_small.tile([P, 4, P], F32, tag="smallps")
                for kk in range(4):
                    et = 4 * eg + kk
                    nc.tensor.matmul(
                        tp[:, kk, :], xn[:, et * P:(et + 1) * P], id_sb,
                        start=True, stop=True)
                dst = xT[:, 4 * eg:4 * eg + 4, it * P:(it + 1) * P]
                if eg == 0:
                    nc.scalar.copy(out=dst, in_=tp)
                else:
                    nc.vector.tensor_copy(out=dst, in_=tp)

        def emit_qk_chunk(b, t, nt):
            # compact q^T/k^T halves; on the last nt of each t, relocate
            # head rows into the 32-aligned region layout.
            s = st8[b]
            if s.get("qkc") is None:
                s["qkc"] = qk_pool.tile([P, 2, N], BF16, tag="qkc", name="qkc")
                s["qT"] = qk_pool.tile([P, 2, N], BF16, tag="qT", name="qT")
                s["kT"] = qk_pool.tile([P, 2, N], BF16, tag="kT", name="kT")
            xT = s["xT"]
            ps = ps_small.tile([P, 512], F32, tag="smallps")
            for et in range(NT):
                nc.tensor.matmul(
                    ps, wqk_sb[:, et, t, :],
                    xT[:, et, nt * 512:(nt + 1) * 512],
                    start=(et == 0), stop=False)
            nc.tensor.matmul(
                ps, bqk_sb[:, t, :], ones512_sb, start=False, stop=True)
            nc.scalar.copy(
                out=s["qkc"][:, t, nt * 512:(nt + 1) * 512], in_=ps)
            if nt == 1:
                dst = s["qT"] if t == 0 else s["kT"]
                for h in range(HEADS):
                    r, c = h // 4, h % 4
                    nc.gpsimd.dma_start(
                        out=dst[32 * c:32 * c + HD, r, :],
                        in_=s["qkc"][HD * h:HD * (h + 1), t, :])

        def emit_v_chunk(b, jt):
            s = st8[b]
            if s.get("v") is None:
                s["v"] = v_pool.tile([P, NT, HEADS, 32], BF16, tag="vt", name="vt")
                nc.gpsimd.memset(s["v"], 0.0)
                nc.gpsimd.memset(s["v"][:, :, :, 0:1], 1.0)
            xT = s["xT"]
            ps = ps_small.tile([P, P], F32, tag="smallps")
            for et in range(NT):
                nc.tensor.matmul(
                    ps, xT[:, et, jt * P:(jt + 1) * P], wv_sb[:, et, :],
                    start=(et == 0), stop=False)
            nc.tensor.matmul(ps, ones1_sb, bv_sb, start=False, stop=True)
            nc.scalar.copy(
                out=s["v"][:, jt, :, 1:17],
                in_=ps[:].rearrange("p (h d) -> p h d", d=16))

        def emit_proj_chunk(b, it, nt):
            s = st8[b]
            ps = ps_small.tile([P, 512], F32, tag="smallps")
            for r in range(2):
                nc.tensor.matmul(
                    ps, s["o"][r][:, it * P:(it + 1) * P],
                    wpj_sb[:, r, nt * 512:(nt + 1) * 512],
                    start=(r == 0), stop=(r == 1))
            fin = fin_pool.tile([P, 512], F32, tag="fin")
            if (it + nt) % 2 == 0:
                nc.scalar.copy(out=fin, in_=ps)
            else:
                nc.vector.tensor_copy(out=fin, in_=ps)
            nc.sync.dma_start(
                out=out_h[b, it * P:(it + 1) * P, nt * 512:(nt + 1) * 512],
                in_=fin)

        def emit_normalize(b, r, ih, oT_ps):
            # divide each head-strip of oT by its softmax row-sum (rows 32c)
            s = st8[b]
            if s["o"][r] is None:
                s["o"][r] = o_pool.tile([P, N], BF16, tag="oT", name="oT")
            i0 = ih * 512
            of = nrm_pool.tile([P, 512], F32, tag="of")
            nc.scalar.copy(out=of, in_=oT_ps)
            rep = nrm_pool.tile([P, 512], F32, tag="rep")
            nc.vector.stream_shuffle(out=rep, in_=of, mask=[0] * 32)
            rrep = nrm_pool.tile([P, 512], F32, tag="rrep")
            nc.vector.reciprocal_approx_fast(out=rrep, in_=rep)
            nc.gpsimd.tensor_tensor(
                out=s["o"][r][:, i0:i0 + 512], in0=of, in1=rrep, op=Mult)

        def emit_attention(b, fillers, fill_mod=2):
            # Software-pipelined: the score matmuls + exp of slot i are
            # emitted two slots ahead of slot i's attn@v matmuls, so the
            # in-order PE queue never head-of-line blocks on an exp.
            s = st8[b]
            s["o"] = [None, None]
            slots = [(ih, r, cp, jt)
                     for ih in range(2) for r in range(2)
                     for cp in range(2) for jt in range(NT)]
            group_ps = {}
            pending = []
            idx = [0]

            def maybe_fill():
                idx[0] += 1
                if fillers and idx[0] % fill_mod == 0:
                    f = fillers.pop(0)
                    if f is not None:
                        f()

            def emit_front(ih, r, cp, jt):
                E = e_pool.tile([P, 2, 512], BF16, tag="E")
                sc = ps_scores.tile([P, 2, 512], F32, tag="sc")
                i0 = ih * 512
                for ci in range(2):
                    c = 2 * cp + ci
                    nc.tensor.matmul(
                        sc[:, ci, :],
                        s["kT"][32 * c:32 * c + 16, r, jt * P:(jt + 1) * P],
                        s["qT"][32 * c:32 * c + 16, r, i0:i0 + 512],
                        start=True, stop=True, tile_position=(32 * c, 0))
                if idx[0] % 3 == 2:
                    nc.vector.tensor_scalar(
                        out=E[:].bitcast(I16), in0=sc,
                        scalar1=A_EXP, scalar2=B_EXP, op0=Mult, op1=Add)
                else:
                    nc.scalar.activation(out=E, in_=sc, func=AF.Exp)
                return E

            def emit_back(ih, r, cp, jt, E):
                if (ih, r) not in group_ps:
                    group_ps[(ih, r)] = ps_out.tile([P, 512], F32, tag="oTps", name="oTps")
                oT_ps = group_ps[(ih, r)]
                for ci in range(2):
                    c = 2 * cp + ci
                    h = 4 * r + c
                    nc.tensor.matmul(
                        oT_ps[32 * c:32 * c + 32, :],
                        s["v"][:, jt, h, :], E[:, ci, :],
                        start=(jt == 0), stop=(jt == NT - 1),
                        tile_position=(0, 32 * c))
                if cp == 1 and jt == NT - 1:
                    emit_normalize(b, r, ih, group_ps.pop((ih, r)))

            for sl in slots:
                E = emit_front(*sl)
                pending.append((sl, E))
                if len(pending) > 2:
                    psl, pE = pending.pop(0)
                    emit_back(*psl, pE)
                maybe_fill()
            for psl, pE in pending:
                emit_back(*psl, pE)

        # ---------- schedule ----------
        st8[0] = {}
        st8[1] = {}
        # preload the exp table while the ramp runs
        dummy = stat.tile([P, 1], F32, tag="dummy")
        nc.scalar.activation(out=dummy, in_=eps_sb, func=AF.Exp)

        def ab_order(b):
            out = []
            for it in range(NT):
                out.append(lambda it=it: emit_x_load(b, it))
                out.append(lambda it=it: emit_ln_stats(b, it))
            out.append(lambda: emit_rsqrt(b))
            for it in range(4):
                out.append(lambda it=it: emit_center(b, it))
                out.append(lambda it=it: emit_ln_tp(b, it))
                out.append(lambda it=it: emit_v_chunk(b, it))
            out.append(lambda: emit_qk_chunk(b, 0, 0))
            out.append(lambda: emit_qk_chunk(b, 1, 0))
            for it in range(4, NT):
                out.append(lambda it=it: emit_center(b, it))
                out.append(lambda it=it: emit_ln_tp(b, it))
                out.append(lambda it=it: emit_v_chunk(b, it))
            out.append(lambda: emit_qk_chunk(b, 0, 1))
            out.append(lambda: emit_qk_chunk(b, 1, 1))
            return out

        for f in ab_order(0):
            f()

        fill_b1 = ab_order(1)
        emit_attention(0, fill_b1, fill_mod=1)
        for f in fill_b1:
            f()

        # proj(0) fills the first half of attention(1); proj(1) columns
        # 0..511 (it 0..3) become available after the ih=0 groups.
        fill_a1 = (
            [lambda it=it, nt=nt: emit_proj_chunk(0, it, nt)
             for it in range(NT) for nt in range(2)]
            + [lambda it=it, nt=nt: emit_proj_chunk(1, it, nt)
               for it in range(4) for nt in range(2)]
        )
        emit_attention(1, fill_a1, fill_mod=3)
        for f in fill_a1:
            f()
        for it in range(4, NT):
            for nt in range(2):
                emit_proj_chunk(1, it, nt)

    nc.finalize()
    return nc


def _prep_weights(gamma, beta, w_qkv, w_proj, b_proj):
    gamma = gamma.astype(np.float64)
    beta = beta.astype(np.float64)
    w_qkv = w_qkv.astype(np.float64)
    w_proj = w_proj.astype(np.float64)
    b_proj = b_proj.astype(np.float64)

    wg = w_qkv * gamma[:, None]
    bias = beta @ w_qkv                   # [384]

    # compact q/k: tile t=0 -> q (SCALE folded), t=1 -> k
    wqk = np.zeros((EMB, 2, P), dtype=np.float64)
    wqk[:, 0, :] = wg[:, :INNER] * SCALE
    wqk[:, 1, :] = wg[:, INNER:2 * INNER]
    bqk = np.zeros((1, 2, P), dtype=np.float64)
    bqk[0, 0, :] = bias[:INNER] * SCALE
    bqk[0, 1, :] = bias[INNER:2 * INNER]
    wqk_t = wqk.reshape(NT, P, 2, P).transpose(1, 0, 2, 3)  # [P, NT, 2, P]

    wv = wg[:, 2 * INNER:3 * INNER].reshape(NT, P, P).transpose(1, 0, 2)
    bv = bias[2 * INNER:3 * INNER].reshape(1, P)

    # o^T row mapping: 32c = ones/rowsum row, 32c+1+d = head (4r+c) dim d
    wpj = np.zeros((P, 2, EMB), dtype=np.float64)
    for r in range(2):
        for c in range(4):
            h = 4 * r + c
            wpj[32 * c + 1:32 * c + 1 + HD, r, :] = \
                w_proj[h * HD:(h + 1) * HD, :]
    wpj[0, 0, :] = b_proj

    bf = ml_dtypes.bfloat16
    return {
        "wqk": np.ascontiguousarray(wqk_t).astype(bf),
        "bqk": np.ascontiguousarray(bqk).astype(bf),
        "wv": np.ascontiguousarray(wv).astype(bf),
        "bv": np.ascontiguousarray(bv).astype(bf),
        "wproj": np.ascontiguousarray(wpj).astype(bf),
        "ident": np.eye(P, dtype=np.float32).astype(bf),
    }


def kernel(x, gamma, beta, w_qkv, w_proj, b_proj):
    if "nc" not in _CACHE:
        _CACHE["nc"] = _build()
    nc = _CACHE["nc"]

    w = _prep_weights(gamma, beta, w_qkv, w_proj, b_proj)
    x = np.asarray(x, dtype=np.float32)
    in_maps = []
    for i in range(NCORES):
        m = {"xs": np.ascontiguousarray(x[i * NB:(i + 1) * NB])}
        m.update(w)
        in_maps.append(m)

    res = run_bass_kernel_spmd(nc, in_maps, core_ids=list(range(NCORES)))
    out = np.concatenate([res.results[i]["out"] for i in range(NCORES)], axis=0)
    return out.astype(np.float32)
